# revision 5
# baseline (speedup 1.0000x reference)
"""Trainium2 Bass kernel for dynamic-scale FP8 GEMM (MixLinear):

    out = (scale_in * scale_w) * (q8(x / scale_in) @ q8(w).T) + bias
    scale_in = max|x| / 448  (global over the whole activation tensor)

Strategy (8 NeuronCores, SPMD, data-parallel over M = B*S = 16384):
  - Each core gets a 2048-row shard of x, full weight + bias (replicated).
  - x is loaded CONTIGUOUSLY ([m-tile, k] layout, fast DMA); per-tile DVE
    abs-max reduces trail the loads so the cross-core AllReduce(max) of the
    [128] per-partition maxima triggers ~26us in -- the collective latency
    is then hidden behind:
      * TensorE 128x128 transposes of x (fp16 -> PSUM -> SBUF [k, m] layout)
      * the weight load+quantize via casting SWDGE DMA (fp16->fp8, [K,N]
        host-pretransposed layout)
  - TRN fp8_e4m3 saturates at +-240 (vs OCP e4m3fn's +-448), so quantize
    with a 2x scale (values land in +-224) and fold the 2x back at dequant.
  - After the collective: global scale -> broadcast -> quantize xT (DVE/Act
    alternating, chunked so the GEMM starts after the first chunk).
  - GEMM in fp8 DoubleRow perf mode (contraction 256/matmul), PSUM evicted
    by ScalarE activation: out = psum*2s + bias (output N-major: psum
    partitions = N-tile, so bias is a per-partition scalar).  Per-core
    output is [N, M_shard]; the host transposes on gather.
"""

import os
import sys

try:
    import concourse  # noqa: F401
except ImportError:  # pragma: no cover
    for _p in ("/opt/trn_rl_repo", "/root/.axon_site/_ro/trn_rl_repo"):
        if os.path.isdir(_p) and _p not in sys.path:
            sys.path.insert(0, _p)

import numpy as np

import concourse.bacc as bacc
import concourse.bass as bass  # noqa: F401
import concourse.mybir as mybir
import concourse.tile as tile
from concourse.bass_utils import run_bass_kernel_spmd

# Problem shapes (hardcoded per contract).
B, S, K, N = 4, 4096, 2048, 2048
M = B * S
N_CORES = 8
MS = M // N_CORES  # 2048 rows of x per core

P = 128
F16 = mybir.dt.float16
F32 = mybir.dt.float32
FP8 = mybir.dt.float8e4


def build_nc(ms=MS, k=K, n=N, n_cores=N_CORES):
    """Build + compile the per-core Bass program (SPMD: same NEFF on all cores)."""
    ko = k // P          # 16 k-planes
    nt_tiles = n // P    # 16 stationary n-tiles
    k_pairs = ko // 2    # 8 DoubleRow k steps
    m_tiles = ms // P    # 16 m-tiles of x
    assert k % 256 == 0 and ms % 1024 == 0 and n % 256 == 0

    nc = bacc.Bacc("TRN2", target_bir_lowering=False, debug=False, num_devices=n_cores)
    x = nc.dram_tensor("x", [ms, k], F16, kind="ExternalInput")
    wt = nc.dram_tensor("wt", [k, n], F16, kind="ExternalInput")
    b = nc.dram_tensor("b", [n], F16, kind="ExternalInput")
    ident = nc.dram_tensor("ident", [P, P], F16, kind="ExternalInput")
    out_t = nc.dram_tensor("out_t", [n, ms], F16, kind="ExternalOutput")

    with tile.TileContext(nc) as tc:
        with (
            tc.tile_pool(name="big", bufs=1) as big,
            tc.tile_pool(name="small", bufs=1) as small,
            tc.tile_pool(name="ev", bufs=6) as ev,
            tc.tile_pool(name="psum", bufs=1, space="PSUM") as psum,
            tc.tile_pool(name="dram", bufs=1, space="DRAM") as dram,
        ):
            # Persistent SBUF tensors.
            xs = big.tile([P, m_tiles, k], F16)  # x natural: partition = m%128
            xT = big.tile([P, ko, ms], F16)      # x transposed: partition = k%128
            xq = big.tile([P, ko, ms], FP8)      # quantized x (scale 2s)
            wq = big.tile([P, ko, n], FP8)       # quantized w (scale 1)

            idn = small.tile([P, P], F16)
            nc.scalar.dma_start(idn[:], ident.ap())
            bias16 = small.tile([P, nt_tiles], F16)
            nc.scalar.dma_start(bias16[:], b.ap().rearrange("(j p) -> p j", p=P))
            bias32 = small.tile([P, nt_tiles], F32)
            nc.gpsimd.tensor_copy(bias32[:], bias16[:])

            # ---- Phase A: contiguous x load + per-tile abs-max --------------
            acc = small.tile([P, m_tiles], F32)
            for mo in range(m_tiles):
                nc.sync.dma_start(xs[:, mo, :], x.ap()[mo * P:(mo + 1) * P, :])
                nc.vector.tensor_reduce(
                    acc[:, mo:mo + 1], xs[:, mo, :],
                    axis=mybir.AxisListType.X, op=mybir.AluOpType.max,
                    apply_absolute_value=True,
                )
            amax_red = small.tile([P, 1], F32)
            nc.vector.tensor_reduce(
                amax_red, acc[:], axis=mybir.AxisListType.X, op=mybir.AluOpType.max,
            )

            # ---- AllReduce(max) of the [128] per-partition maxima -----------
            cc_in = dram.tile([P], F32)
            cc_out = dram.tile([P], F32, addr_space="Shared" if n_cores > 4 else "Local")
            nc.sync.dma_start(cc_in[:], amax_red[:, 0:1])
            nc.gpsimd.collective_compute(
                "AllReduce",
                mybir.AluOpType.max,
                replica_groups=[list(range(n_cores))],
                ins=[cc_in.opt()],
                outs=[cc_out.opt()],
            )

            # ---- Phase W: weight load+quantize via SWDGE cast-DMA -----------
            # wt DRAM is [K, N]; SWDGE casts fp16->fp8e4 (RNE) during the
            # transfer.  k = j*128 + p matches the TensorE-transpose k-mapping
            # used for x.
            for i in range(4):
                n0 = i * (n // 4)
                nc.gpsimd.dma_start(
                    out=wq[:, :, n0:n0 + n // 4],
                    in_=wt.ap()[:, n0:n0 + n // 4].rearrange(
                        "(j p) n2 -> p j n2", p=P
                    ),
                )

            # ---- Phase T: TensorE transposes of x (fp16), chase the loads ---
            # 128x128 blocks; 8 blocks (one k-plane j, 8 consecutive m-tiles)
            # share one full PSUM bank then are evicted in a single copy.
            ei = 0
            for mg in range(m_tiles // 8):
                for j in range(ko):
                    tp = psum.tile(
                        [P, 8 * P], F16, tag="tp", bufs=2, name=f"tp_{mg}_{j}"
                    )
                    for mi in range(8):
                        mo = mg * 8 + mi
                        nc.tensor.matmul(
                            tp[:, mi * P:(mi + 1) * P],
                            lhsT=xs[:, mo, j * P:(j + 1) * P],
                            rhs=idn[:],
                            is_transpose=True,
                            start=(mi == 0),
                            stop=(mi == 7),
                        )
                    dst = xT[:, j, mg * 1024:(mg + 1) * 1024]
                    # GpSimd cannot touch PSUM; split evicts DVE-heavy.
                    if ei % 3 == 1:
                        nc.scalar.copy(dst, tp[:])
                    else:
                        nc.vector.tensor_copy(dst, tp[:])
                    ei += 1

            # ---- Post-collective: global scale + broadcast ------------------
            scal_sb = small.tile([P, P], F32)
            nc.sync.dma_start(scal_sb[0:1, :], cc_out[:])
            gmax = small.tile([P, 1], F32)
            nc.vector.tensor_reduce(
                gmax[0:1, :], scal_sb[0:1, :],
                axis=mybir.AxisListType.X, op=mybir.AluOpType.max,
            )
            ginv = small.tile([P, 1], F32)
            nc.vector.reciprocal(ginv[0:1, :], gmax[0:1, :])
            # sc[:, 0] = inv2s = 224/amax (quant), sc[:, 1] = s2 = amax/224.
            sc_stage = small.tile([P, 2], F32)
            nc.vector.tensor_scalar_mul(sc_stage[0:1, 0:1], ginv[0:1, :], 224.0)
            nc.vector.tensor_scalar_mul(sc_stage[0:1, 1:2], gmax[0:1, :], 1.0 / 224.0)
            sc = small.tile([P, 2], F32)
            nc.gpsimd.partition_broadcast(sc, sc_stage[0:1, :], channels=P)

            # ---- Phase Q: quantize xT -> xq (chunked, DVE/Act alternating) --
            qc = 256
            for c in range(ms // qc):
                sl = slice(c * qc, (c + 1) * qc)
                if c % 2 == 0:
                    nc.vector.tensor_scalar(
                        xq[:, :, sl], xT[:, :, sl], sc[:, 0:1], None,
                        mybir.AluOpType.mult,
                    )
                else:
                    nc.scalar.activation(
                        xq[:, :, sl], xT[:, :, sl],
                        mybir.ActivationFunctionType.Copy, scale=sc[:, 0:1],
                    )

            # ---- Phase G: fp8 DoubleRow GEMM --------------------------------
            for mq in range(ms // 512):
                m0 = mq * 512
                for nt in range(nt_tiles):
                    ps = psum.tile(
                        [P, 512], F32, tag="ps", bufs=6, name=f"ps_{mq}_{nt}"
                    )
                    for k8 in range(k_pairs):
                        nc.tensor.matmul(
                            ps[:],
                            lhsT=wq[:, 2 * k8:2 * k8 + 2, nt * P:(nt + 1) * P],
                            rhs=xq[:, 2 * k8:2 * k8 + 2, m0:m0 + 512],
                            start=(k8 == 0),
                            stop=(k8 == k_pairs - 1),
                            perf_mode=mybir.MatmulPerfMode.DoubleRow,
                        )
                    ob = ev.tile([P, 512], F16, tag="ob", name=f"ob_{mq}_{nt}")
                    nc.scalar.activation(
                        ob[:], ps[:],
                        mybir.ActivationFunctionType.Identity,
                        bias=bias32[:, nt:nt + 1],
                        scale=sc[:, 1:2],
                    )
                    nc.sync.dma_start(
                        out_t.ap()[nt * P:(nt + 1) * P, m0:m0 + 512], ob[:]
                    )

    nc.compile()
    return nc


_NC_CACHE = {}


def _get_nc():
    if "nc" not in _NC_CACHE:
        _NC_CACHE["nc"] = build_nc()
    return _NC_CACHE["nc"]


def kernel(x, weight, bias):
    x = np.asarray(x, dtype=np.float16).reshape(M, K)
    weight = np.asarray(weight, dtype=np.float16)
    bias = np.asarray(bias, dtype=np.float16)

    nc = _get_nc()
    wt = np.ascontiguousarray(weight.T)  # [K, N] -- static-weight layout prep
    ident = np.eye(P, dtype=np.float16)
    in_maps = [
        {"x": x[c * MS:(c + 1) * MS], "wt": wt, "b": bias, "ident": ident}
        for c in range(N_CORES)
    ]
    trace = bool(int(os.environ.get("KERNEL_TRACE", "0")))
    res = run_bass_kernel_spmd(nc, in_maps, list(range(N_CORES)), trace=trace)
    _NC_CACHE["last_result"] = res

    out = np.empty((M, N), dtype=np.float16)
    for c in range(N_CORES):
        out[c * MS:(c + 1) * MS, :] = res.results[c]["out_t"].T
    return out.reshape(B, S, N)


# revision 12
# speedup vs baseline: 1.1724x; 1.1724x over previous
"""Trainium2 Bass kernel for dynamic-scale FP8 GEMM (MixLinear):

    out = (scale_in * scale_w) * (q8(x / scale_in) @ q8(w).T) + bias
    scale_in = max|x| / 448  (global over the whole activation tensor)

Strategy (8 NeuronCores, SPMD, data-parallel over M = B*S = 16384):
  - Each core gets a 2048-row shard of x, full weight + bias (replicated).
  - x is loaded CONTIGUOUSLY ([m-tile, k] layout) split across BOTH HWDGE
    queues (sync+scalar); per-tile DVE abs-max reduces (fp16 accumulators,
    4B-aligned stride-2 columns, to keep the DVE 2x 16-bit mode eligible)
    chase the loads so the cross-core AllReduce(max) of the [128]
    per-partition maxima triggers as early as possible -- the collective
    completes when the LAST core arrives, so every us of local latency
    shifts the whole tail.
  - Hidden inside the collective window:
      * TensorE 128x128 transposes of x (fp16 -> PSUM, ScalarE evicts to
        SBUF [k, m] layout)
      * the weight load+quantize via casting SWDGE DMA (fp16->fp8, [K,N]
        host-pretransposed), ordered AFTER the x loads so the amax-critical
        phase gets the full HBM bandwidth.
  - TRN fp8_e4m3 saturates at +-240 (vs OCP e4m3fn's +-448), so quantize
    with a 2x scale (values land in +-224) and fold the 2x back at dequant.
  - After the collective: global scale -> broadcast -> DVE quantizes xT
    in m-chunks so the GEMM starts after the first chunk.
  - GEMM in fp8 DoubleRow perf mode (contraction 256/matmul), PSUM evicted
    by ScalarE activation: out = psum*2s + bias (output N-major: psum
    partitions = N-tile, so bias is a per-partition scalar).  Per-core
    output is [N, M_shard]; the host transposes on gather.
"""

import os
import sys

try:
    import concourse  # noqa: F401
except ImportError:  # pragma: no cover
    for _p in ("/opt/trn_rl_repo", "/root/.axon_site/_ro/trn_rl_repo"):
        if os.path.isdir(_p) and _p not in sys.path:
            sys.path.insert(0, _p)

import numpy as np

import concourse.bacc as bacc
import concourse.bass as bass  # noqa: F401
import concourse.mybir as mybir
import concourse.tile as tile
from concourse import bass_isa
from concourse.bass_utils import run_bass_kernel_spmd

# Problem shapes (hardcoded per contract).
B, S, K, N = 4, 4096, 2048, 2048
M = B * S
N_CORES = 8
MS = M // N_CORES  # 2048 rows of x per core

P = 128
F16 = mybir.dt.float16
F32 = mybir.dt.float32
FP8 = mybir.dt.float8e4


def build_nc(ms=MS, k=K, n=N, n_cores=N_CORES):
    """Build + compile the per-core Bass program (SPMD: same NEFF on all cores)."""
    ko = k // P          # 16 k-planes
    nt_tiles = n // P    # 16 stationary n-tiles
    k_pairs = ko // 2    # 8 DoubleRow k steps
    m_tiles = ms // P    # 16 m-tiles of x
    assert k % 256 == 0 and ms % 1024 == 0 and n % 256 == 0

    nc = bacc.Bacc("TRN2", target_bir_lowering=False, debug=False, num_devices=n_cores)
    x = nc.dram_tensor("x", [ms, k], F16, kind="ExternalInput")
    wt = nc.dram_tensor("wt", [k, n], F16, kind="ExternalInput")
    b = nc.dram_tensor("b", [n], F16, kind="ExternalInput")
    ident = nc.dram_tensor("ident", [P, P], F16, kind="ExternalInput")
    out_t = nc.dram_tensor("out_t", [n, ms], F16, kind="ExternalOutput")

    with tile.TileContext(nc) as tc:
        with (
            tc.tile_pool(name="big", bufs=1) as big,
            tc.tile_pool(name="small", bufs=1) as small,
            tc.tile_pool(name="ev", bufs=6) as ev,
            tc.tile_pool(name="psum", bufs=1, space="PSUM") as psum,
            tc.tile_pool(name="dram", bufs=1, space="DRAM") as dram,
        ):
            # Persistent SBUF tensors.
            xs = big.tile([P, m_tiles, k], F16)  # x natural: partition = m%128
            xT = big.tile([P, ko, ms], F16)      # x transposed: partition = k%128
            xq = big.tile([P, ko, ms], FP8)      # quantized x (scale 2s)
            wq = big.tile([P, ko, n], FP8)       # quantized w (scale 1)

            idn = small.tile([P, P], F16)
            nc.scalar.dma_start(idn[:], ident.ap())
            bias16 = small.tile([P, nt_tiles], F16)
            nc.scalar.dma_start(bias16[:], b.ap().rearrange("(j p) -> p j", p=P))
            bias32 = small.tile([P, nt_tiles], F32)
            nc.gpsimd.tensor_copy(bias32[:], bias16[:])

            # ---- Phase A: x load (both HWDGE queues) + per-tile abs-max -----
            # Reduce each tile as [128, 2, 1024] -> [128, 2] fp16: with a
            # 2-elem 4B-aligned dst and all-2-byte operands the DVE 2x 16-bit
            # perf mode stays eligible (a 1-elem dst disqualifies it).
            acc16 = small.tile([P, 2 * m_tiles], F16)
            x_loads = []
            for mo in range(m_tiles):
                q = nc.sync if mo % 2 == 0 else nc.scalar
                ld = q.dma_start(xs[:, mo, :], x.ap()[mo * P:(mo + 1) * P, :])
                x_loads.append(ld)
                nc.vector.tensor_reduce(
                    acc16[:, 2 * mo:2 * mo + 2],
                    xs[:, mo, :].rearrange("p (h q) -> p h q", h=2),
                    axis=mybir.AxisListType.X, op=mybir.AluOpType.max,
                    apply_absolute_value=True,
                )
            amax_red = small.tile([P, 1], F32)
            nc.vector.tensor_reduce(
                amax_red, acc16[:], axis=mybir.AxisListType.X, op=mybir.AluOpType.max,
            )
            amax_all = small.tile([P, 1], F32)
            nc.gpsimd.partition_all_reduce(
                amax_all, amax_red, channels=P, reduce_op=bass_isa.ReduceOp.max
            )

            # ---- AllReduce(max) of the scalar per-core maxima ---------------
            # Single-element payload: the SBUF->DRAM staging is ONE descriptor
            # (a [128,1] column costs 128 4-byte descriptors, ~15us).
            cc_in = dram.tile([1], F32)
            cc_out = dram.tile([1], F32, addr_space="Shared" if n_cores > 4 else "Local")
            nc.gpsimd.dma_start(cc_in[:], amax_all[0:1, 0:1])
            nc.gpsimd.collective_compute(
                "AllReduce",
                mybir.AluOpType.max,
                replica_groups=[list(range(n_cores))],
                ins=[cc_in.opt()],
                outs=[cc_out.opt()],
            )

            # ---- Phase W: weight load+quantize via SWDGE cast-DMA -----------
            # wt DRAM is [K, N]; SWDGE casts fp16->fp8e4 (RNE) during the
            # transfer.  k = j*128 + p matches the TensorE-transpose k-mapping
            # used for x.  Ordered after the x loads: the amax chain owns HBM
            # first; chunk i is only needed ~8.5us into the GEMM per n-range.
            for i in range(4):
                n0 = i * (n // 4)
                wi = nc.gpsimd.dma_start(
                    out=wq[:, :, n0:n0 + n // 4],
                    in_=wt.ap()[:, n0:n0 + n // 4].rearrange(
                        "(j p) n2 -> p j n2", p=P
                    ),
                )
                tile.add_dep_helper(
                    wi.ins, x_loads[-1].ins,
                    reason="defer weight HBM traffic behind the amax-critical x load",
                )

            # ---- Phase T: TensorE transposes of x (fp16), chase the loads ---
            # 128x128 blocks; 8 blocks (one k-plane j, 8 consecutive m-tiles)
            # share one full PSUM bank then ScalarE evicts them in one copy.
            for mg in range(m_tiles // 8):
                for j in range(ko):
                    tp = psum.tile(
                        [P, 8 * P], F16, tag="tp", bufs=3, name=f"tp_{mg}_{j}"
                    )
                    for mi in range(8):
                        mo = mg * 8 + mi
                        nc.tensor.matmul(
                            tp[:, mi * P:(mi + 1) * P],
                            lhsT=xs[:, mo, j * P:(j + 1) * P],
                            rhs=idn[:],
                            is_transpose=True,
                            start=(mi == 0),
                            stop=(mi == 7),
                        )
                    nc.scalar.copy(xT[:, j, mg * 1024:(mg + 1) * 1024], tp[:])

            # ---- Post-collective: global scale + broadcast ------------------
            gmax = small.tile([P, 1], F32)
            nc.sync.dma_start(gmax[0:1, :], cc_out[:])
            ginv = small.tile([P, 1], F32)
            nc.vector.reciprocal(ginv[0:1, :], gmax[0:1, :])
            # sc[:, 0] = inv2s = 224/amax (quant), sc[:, 1] = s2 = amax/224.
            sc_stage = small.tile([P, 2], F32)
            nc.vector.tensor_scalar_mul(sc_stage[0:1, 0:1], ginv[0:1, :], 224.0)
            nc.vector.tensor_scalar_mul(sc_stage[0:1, 1:2], gmax[0:1, :], 1.0 / 224.0)
            sc = small.tile([P, 2], F32)
            nc.gpsimd.partition_broadcast(sc, sc_stage[0:1, :], channels=P)

            # ---- Phase Q: DVE quantizes xT -> xq in m-chunks ----------------
            # Small first chunk so the first GEMM m-quarter unblocks sooner.
            edges = [0, 128, 256] + list(range(512, ms + 1, 256))
            for c0, c1 in zip(edges, edges[1:]):
                nc.vector.tensor_scalar(
                    xq[:, :, c0:c1], xT[:, :, c0:c1], sc[:, 0:1], None,
                    mybir.AluOpType.mult,
                )

            # ---- Phase G: fp8 DoubleRow GEMM --------------------------------
            for mq in range(ms // 512):
                m0 = mq * 512
                for nt in range(nt_tiles):
                    ps = psum.tile(
                        [P, 512], F32, tag="ps", bufs=5, name=f"ps_{mq}_{nt}"
                    )
                    for k8 in range(k_pairs):
                        nc.tensor.matmul(
                            ps[:],
                            lhsT=wq[:, 2 * k8:2 * k8 + 2, nt * P:(nt + 1) * P],
                            rhs=xq[:, 2 * k8:2 * k8 + 2, m0:m0 + 512],
                            start=(k8 == 0),
                            stop=(k8 == k_pairs - 1),
                            perf_mode=mybir.MatmulPerfMode.DoubleRow,
                        )
                    ob = ev.tile([P, 512], F16, tag="ob", name=f"ob_{mq}_{nt}")
                    nc.scalar.activation(
                        ob[:], ps[:],
                        mybir.ActivationFunctionType.Identity,
                        bias=bias32[:, nt:nt + 1],
                        scale=sc[:, 1:2],
                    )
                    oq = nc.sync if nt % 2 == 0 else nc.scalar
                    oq.dma_start(
                        out_t.ap()[nt * P:(nt + 1) * P, m0:m0 + 512], ob[:]
                    )

    nc.compile()
    return nc


_NC_CACHE = {}


def _get_nc():
    if "nc" not in _NC_CACHE:
        _NC_CACHE["nc"] = build_nc()
    return _NC_CACHE["nc"]


def kernel(x, weight, bias):
    x = np.asarray(x, dtype=np.float16).reshape(M, K)
    weight = np.asarray(weight, dtype=np.float16)
    bias = np.asarray(bias, dtype=np.float16)

    nc = _get_nc()
    wt = np.ascontiguousarray(weight.T)  # [K, N] -- static-weight layout prep
    ident = np.eye(P, dtype=np.float16)
    in_maps = [
        {"x": x[c * MS:(c + 1) * MS], "wt": wt, "b": bias, "ident": ident}
        for c in range(N_CORES)
    ]
    trace = bool(int(os.environ.get("KERNEL_TRACE", "0")))
    res = run_bass_kernel_spmd(nc, in_maps, list(range(N_CORES)), trace=trace)
    _NC_CACHE["last_result"] = res

    out = np.empty((M, N), dtype=np.float16)
    for c in range(N_CORES):
        out[c * MS:(c + 1) * MS, :] = res.results[c]["out_t"].T
    return out.reshape(B, S, N)


# revision 16
# speedup vs baseline: 1.1929x; 1.0175x over previous
"""Trainium2 Bass kernel for dynamic-scale FP8 GEMM (MixLinear):

    out = (scale_in * scale_w) * (q8(x / scale_in) @ q8(w).T) + bias
    scale_in = max|x| / 448  (global over the whole activation tensor)

Strategy (8 NeuronCores, SPMD, data-parallel over M = B*S = 16384):
  - Each core gets a 2048-row shard of x, full weight + bias (replicated).
  - x is loaded CONTIGUOUSLY ([m-tile, k] layout) split across BOTH HWDGE
    queues (sync+scalar); per-tile DVE abs-max reduces (fp16 accumulators,
    4B-aligned stride-2 columns, to keep the DVE 2x 16-bit mode eligible)
    chase the loads so the cross-core AllReduce(max) of the [128]
    per-partition maxima triggers as early as possible -- the collective
    completes when the LAST core arrives, so every us of local latency
    shifts the whole tail.
  - Hidden inside the collective window:
      * TensorE 128x128 transposes of x (fp16 -> PSUM, ScalarE evicts to
        SBUF [k, m] layout)
      * the weight load+quantize via casting SWDGE DMA (fp16->fp8, [K,N]
        host-pretransposed), ordered AFTER the x loads so the amax-critical
        phase gets the full HBM bandwidth.
  - TRN fp8_e4m3 saturates at +-240 (vs OCP e4m3fn's +-448), so quantize
    with a 2x scale (values land in +-224) and fold the 2x back at dequant.
  - After the collective: global scale -> broadcast -> DVE quantizes xT
    in m-chunks so the GEMM starts after the first chunk.
  - GEMM in fp8 DoubleRow perf mode (contraction 256/matmul), PSUM evicted
    by ScalarE activation: out = psum*2s + bias (output N-major: psum
    partitions = N-tile, so bias is a per-partition scalar).  Per-core
    output is [N, M_shard]; the host transposes on gather.
"""

import os
import sys

try:
    import concourse  # noqa: F401
except ImportError:  # pragma: no cover
    for _p in ("/opt/trn_rl_repo", "/root/.axon_site/_ro/trn_rl_repo"):
        if os.path.isdir(_p) and _p not in sys.path:
            sys.path.insert(0, _p)

import numpy as np

import concourse.bacc as bacc
import concourse.bass as bass  # noqa: F401
import concourse.mybir as mybir
import concourse.tile as tile
from concourse import bass_isa
from concourse.bass_utils import run_bass_kernel_spmd

# Problem shapes (hardcoded per contract).
B, S, K, N = 4, 4096, 2048, 2048
M = B * S
N_CORES = 8
MS = M // N_CORES  # 2048 rows of x per core

P = 128
F16 = mybir.dt.float16
F32 = mybir.dt.float32
FP8 = mybir.dt.float8e4


def build_nc(ms=MS, k=K, n=N, n_cores=N_CORES):
    """Build + compile the per-core Bass program (SPMD: same NEFF on all cores)."""
    ko = k // P          # 16 k-planes
    nt_tiles = n // P    # 16 stationary n-tiles
    k_pairs = ko // 2    # 8 DoubleRow k steps
    m_tiles = ms // P    # 16 m-tiles of x
    assert k % 256 == 0 and ms % 1024 == 0 and n % 256 == 0

    nc = bacc.Bacc("TRN2", target_bir_lowering=False, debug=False, num_devices=n_cores)
    x = nc.dram_tensor("x", [ms, k], F16, kind="ExternalInput")
    wt = nc.dram_tensor("wt", [k, n], F16, kind="ExternalInput")
    b = nc.dram_tensor("b", [n], F16, kind="ExternalInput")
    ident = nc.dram_tensor("ident", [P, P], F16, kind="ExternalInput")
    out_t = nc.dram_tensor("out_t", [n, ms], F16, kind="ExternalOutput")

    with tile.TileContext(nc) as tc:
        with (
            tc.tile_pool(name="big", bufs=1) as big,
            tc.tile_pool(name="small", bufs=1) as small,
            tc.tile_pool(name="ev", bufs=6) as ev,
            tc.tile_pool(name="psum", bufs=1, space="PSUM") as psum,
            tc.tile_pool(name="dram", bufs=1, space="DRAM") as dram,
        ):
            # Persistent SBUF tensors.
            xs = big.tile([P, m_tiles, k], F16)  # x natural: partition = m%128
            xT = big.tile([P, ko, ms], F16)      # x transposed: partition = k%128
            xq = big.tile([P, ko, ms], FP8)      # quantized x (scale 2s)
            wq = big.tile([P, ko, n], FP8)       # quantized w (scale 1)

            idn = small.tile([P, P], F16)
            nc.scalar.dma_start(idn[:], ident.ap())
            bias16 = small.tile([P, nt_tiles], F16)
            nc.scalar.dma_start(bias16[:], b.ap().rearrange("(j p) -> p j", p=P))
            bias32 = small.tile([P, nt_tiles], F32)
            nc.gpsimd.tensor_copy(bias32[:], bias16[:])

            # ---- Phase A: x load (both HWDGE queues) + per-tile abs-max -----
            # Reduce each tile as [128, 2, 1024] -> [128, 2] fp16: with a
            # 2-elem 4B-aligned dst and all-2-byte operands the DVE 2x 16-bit
            # perf mode stays eligible (a 1-elem dst disqualifies it).
            acc16 = small.tile([P, 2 * m_tiles], F16)
            x_loads = []
            x_queues = [nc.sync, nc.scalar, nc.gpsimd]
            for mo in range(m_tiles):
                q = x_queues[mo % 3]
                ld = q.dma_start(xs[:, mo, :], x.ap()[mo * P:(mo + 1) * P, :])
                x_loads.append(ld)
                nc.vector.tensor_reduce(
                    acc16[:, 2 * mo:2 * mo + 2],
                    xs[:, mo, :].rearrange("p (h q) -> p h q", h=2),
                    axis=mybir.AxisListType.X, op=mybir.AluOpType.max,
                    apply_absolute_value=True,
                )
            amax_red = small.tile([P, 1], F32)
            nc.vector.tensor_reduce(
                amax_red, acc16[:], axis=mybir.AxisListType.X, op=mybir.AluOpType.max,
            )
            amax_all = small.tile([P, 1], F32)
            nc.gpsimd.partition_all_reduce(
                amax_all, amax_red, channels=P, reduce_op=bass_isa.ReduceOp.max
            )

            # ---- AllReduce(max) of the scalar per-core maxima ---------------
            # Single-element payload: the SBUF->DRAM staging is ONE descriptor
            # (a [128,1] column costs 128 4-byte descriptors, ~15us).
            cc_in = dram.tile([1], F32)
            cc_out = dram.tile([1], F32, addr_space="Shared" if n_cores > 4 else "Local")
            nc.gpsimd.dma_start(cc_in[:], amax_all[0:1, 0:1])
            cc = nc.gpsimd.collective_compute(
                "AllReduce",
                mybir.AluOpType.max,
                replica_groups=[list(range(n_cores))],
                ins=[cc_in.opt()],
                outs=[cc_out.opt()],
            )

            # ---- Phase W: weight load+quantize via SWDGE cast-DMA -----------
            # wt DRAM is [K, N]; SWDGE casts fp16->fp8e4 (RNE) during the
            # transfer.  k = j*128 + p matches the TensorE-transpose k-mapping
            # used for x.  MUST be ordered after the collective trigger: the
            # cc trigger fences behind all outstanding DMA on the engine, so
            # weight transfers issued first delay the AllReduce by ~15-30us.
            # 8 chunks so each n-range lands before the GEMM consumes it.
            for i in range(8):
                n0 = i * (n // 8)
                wi = nc.gpsimd.dma_start(
                    out=wq[:, :, n0:n0 + n // 8],
                    in_=wt.ap()[:, n0:n0 + n // 8].rearrange(
                        "(j p) n2 -> p j n2", p=P
                    ),
                )
                tile.add_dep_helper(
                    wi.ins, cc.ins,
                    reason="weight SWDGE traffic must not fence the cc trigger",
                )

            # ---- Phase T: TensorE transposes of x (fp16), chase the loads ---
            # 128x128 blocks; 8 blocks (one k-plane j, 8 consecutive m-tiles)
            # share one full PSUM bank then ScalarE evicts them in one copy.
            for mg in range(m_tiles // 8):
                for j in range(ko):
                    tp = psum.tile(
                        [P, 8 * P], F16, tag="tp", bufs=3, name=f"tp_{mg}_{j}"
                    )
                    for mi in range(8):
                        mo = mg * 8 + mi
                        nc.tensor.matmul(
                            tp[:, mi * P:(mi + 1) * P],
                            lhsT=xs[:, mo, j * P:(j + 1) * P],
                            rhs=idn[:],
                            is_transpose=True,
                            start=(mi == 0),
                            stop=(mi == 7),
                        )
                    nc.scalar.copy(xT[:, j, mg * 1024:(mg + 1) * 1024], tp[:])

            # ---- Post-collective: broadcast first, then per-partition math --
            # (one fewer cross-engine hop than compute-then-broadcast)
            gmax = small.tile([P, 1], F32)
            nc.sync.dma_start(gmax[0:1, :], cc_out[:])
            gmax_bc = small.tile([P, 1], F32)
            nc.gpsimd.partition_broadcast(gmax_bc, gmax[0:1, :], channels=P)
            ginv = small.tile([P, 1], F32)
            nc.vector.reciprocal(ginv, gmax_bc)
            # sc[:, 0] = inv2s = 224/amax (quant), sc[:, 1] = s2 = amax/224.
            sc = small.tile([P, 2], F32)
            nc.vector.tensor_scalar_mul(sc[:, 0:1], ginv, 224.0)
            nc.vector.tensor_scalar_mul(sc[:, 1:2], gmax_bc, 1.0 / 224.0)

            # ---- Phase Q: DVE quantizes xT -> xq in m-chunks ----------------
            # Small first chunk so the first GEMM m-quarter unblocks sooner.
            edges = [0, 128, 256] + list(range(512, ms + 1, 256))
            for c0, c1 in zip(edges, edges[1:]):
                nc.vector.tensor_scalar(
                    xq[:, :, c0:c1], xT[:, :, c0:c1], sc[:, 0:1], None,
                    mybir.AluOpType.mult,
                )

            # ---- Phase G: fp8 DoubleRow GEMM --------------------------------
            for mq in range(ms // 512):
                m0 = mq * 512
                for nt in range(nt_tiles):
                    ps = psum.tile(
                        [P, 512], F32, tag="ps", bufs=5, name=f"ps_{mq}_{nt}"
                    )
                    for k8 in range(k_pairs):
                        nc.tensor.matmul(
                            ps[:],
                            lhsT=wq[:, 2 * k8:2 * k8 + 2, nt * P:(nt + 1) * P],
                            rhs=xq[:, 2 * k8:2 * k8 + 2, m0:m0 + 512],
                            start=(k8 == 0),
                            stop=(k8 == k_pairs - 1),
                            perf_mode=mybir.MatmulPerfMode.DoubleRow,
                        )
                    ob = ev.tile([P, 512], F16, tag="ob", name=f"ob_{mq}_{nt}")
                    nc.scalar.activation(
                        ob[:], ps[:],
                        mybir.ActivationFunctionType.Identity,
                        bias=bias32[:, nt:nt + 1],
                        scale=sc[:, 1:2],
                    )
                    # Last m-quarter drains 3-way so the final store tail is
                    # short; earlier quarters stay off the (weight-busy) SWDGE.
                    if mq == ms // 512 - 1:
                        oq = [nc.sync, nc.scalar, nc.gpsimd][nt % 3]
                    else:
                        oq = nc.sync if nt % 2 == 0 else nc.scalar
                    oq.dma_start(
                        out_t.ap()[nt * P:(nt + 1) * P, m0:m0 + 512], ob[:]
                    )

    nc.compile()
    return nc


_NC_CACHE = {}


def _get_nc():
    if "nc" not in _NC_CACHE:
        _NC_CACHE["nc"] = build_nc()
    return _NC_CACHE["nc"]


def kernel(x, weight, bias):
    x = np.asarray(x, dtype=np.float16).reshape(M, K)
    weight = np.asarray(weight, dtype=np.float16)
    bias = np.asarray(bias, dtype=np.float16)

    nc = _get_nc()
    wt = np.ascontiguousarray(weight.T)  # [K, N] -- static-weight layout prep
    ident = np.eye(P, dtype=np.float16)
    in_maps = [
        {"x": x[c * MS:(c + 1) * MS], "wt": wt, "b": bias, "ident": ident}
        for c in range(N_CORES)
    ]
    trace = bool(int(os.environ.get("KERNEL_TRACE", "0")))
    res = run_bass_kernel_spmd(nc, in_maps, list(range(N_CORES)), trace=trace)
    _NC_CACHE["last_result"] = res

    out = np.empty((M, N), dtype=np.float16)
    for c in range(N_CORES):
        out[c * MS:(c + 1) * MS, :] = res.results[c]["out_t"].T
    return out.reshape(B, S, N)
